# revision 1
# baseline (speedup 1.0000x reference)
"""GNN message-passing (std aggregator) on 8 TRN2 NeuronCores.

Math per target node: count, S1 = sum x[src], S2 = sum x[src]^2;
mean = S1/max(count,eps); var = S2/count - mean^2;
std = sqrt(max(var,0)), zeroed where count <= 1.

Strategy: shard TARGET nodes across cores (no collectives). Host packs nodes
into 128-bin blocks balanced by in-degree (serpentine deal), buckets edges by
(block, src-quarter) with uniform tile capacity tq per (block,quarter) so one
NEFF serves all cores. Device per core, per group of GB blocks:
  - 4x dma_gather (one per src quarter of x; int16 idx < 25000) pulls
    x[src] rows into SBUF in quarter-major column order,
  - ACT builds rhs tiles [x | x^2 | 1] (cast to MM dtype),
  - DVE builds 4-wide one-hot tiles (slot-vs-iota is_equal),
  - PE matmul-accumulates [128 bins x 129] = [S1 | S2 | count] in PSUM,
  - small DVE/ACT finishing pass computes std, DMA out per block.
"""

import numpy as np

N_NODES = 100000
N_FEAT = 64
N_EDGES = 1600000
P = 128
NCORES = 8
NB = 98                 # blocks per core
NBLK = NCORES * NB      # 784
GB = 7                  # blocks per group; 98 = 14*7
NQUART = 4
NQ = N_NODES // NQUART  # rows per src quarter (25000 < 32768 for int16 idx)
EPS = 1e-8
MM_DT = "bfloat16"      # "float32" | "bfloat16" for matmul operands

_CACHE = {}


def _build_program(n_nodes, f, nb, tq, gb, nq, mm_dt):
    import concourse.bass as bass
    import concourse.bacc as bacc
    import concourse.mybir as mybir
    import concourse.tile as tile

    F32 = mybir.dt.float32
    I16 = mybir.dt.int16
    MDT = getattr(mybir.dt, mm_dt)
    AO = mybir.AluOpType
    AF = mybir.ActivationFunctionType

    t = NQUART * tq            # tiles (columns) per block
    W = 2 * f + 1              # 129
    C = nb * t                 # total columns per core
    gcols = gb * t             # columns per group
    qcols = gb * tq            # columns per (group, quarter)
    ng = nb // gb
    nidx = qcols * P           # indices per gather
    i16c = nidx // 16          # idx16 cols per gather

    nc = bacc.Bacc()
    xd = nc.declare_dram_parameter("x", [n_nodes, f], F32, isOutput=False)
    gidxd = nc.declare_dram_parameter(
        "gidx", [P, ng * NQUART * i16c], I16, isOutput=False)
    tgtd = nc.declare_dram_parameter("tgt", [P, C], F32, isOutput=False)
    outd = nc.declare_dram_parameter("out", [nb * P, f], F32, isOutput=True)

    with tile.TileContext(nc) as tc:
        with (
            tc.tile_pool(name="const", bufs=1) as constp,
            tc.tile_pool(name="io", bufs=2) as iop,
            tc.tile_pool(name="msg", bufs=2) as msgp,
            tc.tile_pool(name="oh", bufs=6) as ohp,
            tc.tile_pool(name="fin", bufs=4) as finp,
            tc.tile_pool(name="ov", bufs=4) as ovp,
            tc.tile_pool(name="ps", bufs=8, space="PSUM") as psump,
        ):
            # 4-wide iota [128, 4*128]: value = column index % 128
            iota4 = constp.tile([P, 4 * P], F32)
            nc.gpsimd.iota(iota4[:], pattern=[[0, 4], [1, P]], base=0,
                           channel_multiplier=0,
                           allow_small_or_imprecise_dtypes=True)

            for g in range(ng):
                idx = iop.tile([P, NQUART * i16c], I16, tag="idx")
                tg = iop.tile([P, gcols], F32, tag="tg")
                nc.sync.dma_start(
                    out=idx[:],
                    in_=gidxd[:, g * NQUART * i16c:(g + 1) * NQUART * i16c])
                nc.sync.dma_start(
                    out=tg[:], in_=tgtd[:, g * gcols:(g + 1) * gcols])
                tgv = iop.tile([P, gcols], F32, tag="tgv")
                nc.vector.tensor_copy(out=tgv[:], in_=tg[:])

                gbuf = msgp.tile([P, gcols * f], F32, tag="g")
                g3 = gbuf[:].rearrange("p (c e) -> p c e", e=f)
                for qq in range(NQUART):
                    nc.gpsimd.dma_gather(
                        out_ap=g3[:, qq * qcols:(qq + 1) * qcols, :],
                        in_ap=xd[qq * nq:(qq + 1) * nq, :],
                        idxs_ap=idx[:, qq * i16c:(qq + 1) * i16c],
                        num_idxs=nidx,
                        num_idxs_reg=nidx,
                        elem_size=f,
                        single_packet=False,
                    )
                sqx = msgp.tile([P, gcols * W], MDT, tag="sqx")
                s3 = sqx[:].rearrange("p (c w) -> p c w", w=W)
                nc.scalar.activation(out=s3[:, :, 0:f], in_=g3[:, :, :],
                                     func=AF.Copy)
                nc.scalar.square(out=s3[:, :, f:2 * f], in_=g3[:, :, :])
                nc.scalar.activation(out=s3[:, :, 2 * f:W], in_=g3[:, :, 0:1],
                                     func=AF.Copy, bias=1.0, scale=0.0)

                pss = [psump.tile([P, W], F32, tag="ps", name=f"ps_{g}_{bl}")
                       for bl in range(gb)]
                for pk in range(gcols // 4):
                    oh4 = ohp.tile([P, 4 * P], MDT)
                    nc.vector.tensor_tensor(
                        out=oh4[:].rearrange("p (c e) -> p c e", e=P),
                        in0=tgv[:, 4 * pk:4 * pk + 4]
                            .rearrange("p (c u) -> p c u", u=1)
                            .to_broadcast([P, 4, P]),
                        in1=iota4[:].rearrange("p (c e) -> p c e", e=P),
                        op=AO.is_equal,
                    )
                    for i in range(4):
                        cl = 4 * pk + i
                        qq = cl // qcols
                        r = cl % qcols
                        bl = r // tq
                        j = r % tq
                        nc.tensor.matmul(
                            out=pss[bl][:],
                            lhsT=oh4[:, i * P:(i + 1) * P],
                            rhs=sqx[:, cl * W:(cl + 1) * W],
                            start=(qq == 0 and j == 0),
                            stop=(qq == NQUART - 1 and j == tq - 1),
                        )
                for bl in range(gb):
                    b = g * gb + bl
                    ps = pss[bl]
                    cnt = finp.tile([P, 1], F32, tag="cnt")
                    nc.vector.tensor_scalar(
                        out=cnt[:], in0=ps[:, 2 * f:W],
                        scalar1=float(EPS), scalar2=None, op0=AO.max)
                    rec = finp.tile([P, 1], F32, tag="rec")
                    nc.vector.reciprocal(out=rec[:], in_=cnt[:])
                    mean = finp.tile([P, f], F32, tag="mean")
                    nc.vector.tensor_scalar_mul(
                        out=mean[:], in0=ps[:, 0:f], scalar1=rec[:])
                    ex2 = finp.tile([P, f], F32, tag="ex2")
                    nc.vector.tensor_scalar_mul(
                        out=ex2[:], in0=ps[:, f:2 * f], scalar1=rec[:])
                    var = finp.tile([P, f], F32, tag="var")
                    nc.vector.tensor_tensor(
                        out=var[:], in0=mean[:], in1=mean[:], op=AO.mult)
                    nc.vector.tensor_tensor(
                        out=var[:], in0=ex2[:], in1=var[:], op=AO.subtract)
                    nc.vector.tensor_scalar(
                        out=var[:], in0=var[:], scalar1=0.0, scalar2=None,
                        op0=AO.max)
                    std = ovp.tile([P, f], F32, tag="std")
                    nc.scalar.sqrt(out=std[:], in_=var[:])
                    mask = finp.tile([P, 1], F32, tag="mask")
                    nc.vector.tensor_scalar(
                        out=mask[:], in0=ps[:, 2 * f:W],
                        scalar1=1.5, scalar2=None, op0=AO.is_gt)
                    nc.vector.tensor_scalar_mul(
                        out=std[:], in0=std[:], scalar1=mask[:])
                    nc.sync.dma_start(
                        out=outd[b * P:(b + 1) * P, :], in_=std[:])
    return nc


def _host_prep(x, edge_index):
    src = np.asarray(edge_index[0], dtype=np.int64)
    tgt = np.asarray(edge_index[1], dtype=np.int64)
    n_edges = src.shape[0]
    counts = np.bincount(tgt, minlength=N_NODES)

    # serpentine deal of count-sorted nodes into NBLK blocks of <=128 slots
    order = np.argsort(-counts, kind="stable")
    ranks = np.arange(N_NODES)
    rounds = ranks // NBLK
    pos = ranks % NBLK
    blk_of_rank = np.where(rounds % 2 == 0, pos, NBLK - 1 - pos)
    blk = np.empty(N_NODES, np.int64)
    slot = np.empty(N_NODES, np.int64)
    blk[order] = blk_of_rank
    slot[order] = rounds
    assert slot.max() < P

    eb = blk[tgt]                      # edge -> block
    eq = src // NQ                     # edge -> src quarter
    es = slot[tgt]                     # edge -> slot in block
    seg = eb * NQUART + eq             # edge -> (block, quarter) segment
    segsums = np.bincount(seg, minlength=NBLK * NQUART)
    tq = int(np.ceil(segsums.max() / P))
    cap = tq * P

    order_e = np.argsort(seg, kind="stable")
    segs = seg[order_e]
    starts = np.zeros(NBLK * NQUART, np.int64)
    np.cumsum(segsums[:-1], out=starts[1:])
    within = np.arange(n_edges) - starts[segs]
    flat = segs * cap + within

    gidxq = np.zeros((NBLK, NQUART, cap), np.int16)
    tgtq = np.full((NBLK, NQUART, cap), -1.0, np.float32)
    gidxq.reshape(-1)[flat] = (src[order_e] % NQ).astype(np.int16)
    tgtq.reshape(-1)[flat] = es[order_e].astype(np.float32)

    xf = np.ascontiguousarray(np.asarray(x, dtype=np.float32))
    ng = NB // GB
    i16c = GB * cap // 16

    in_maps = []
    for c in range(NCORES):
        tb = tgtq[c * NB:(c + 1) * NB]          # [NB, 4, cap]
        gi = gidxq[c * NB:(c + 1) * NB]
        # tgt columns: (group, quarter, block, tile) -> [P, C]
        tcore = (tb.reshape(ng, GB, NQUART, cap)
                 .transpose(0, 2, 1, 3)          # [ng, 4, GB, cap]
                 .reshape(ng * NQUART * GB * tq, P).T)
        # idx16: per (group, quarter): stream of GB*cap idxs wrapped %16
        gs = (gi.reshape(ng, GB, NQUART, cap)
              .transpose(0, 2, 1, 3)             # [ng, 4, GB, cap]
              .reshape(ng * NQUART, GB * cap))   # per-gather streams
        idx16 = np.ascontiguousarray(
            np.tile(gs.reshape(ng * NQUART, i16c, 16).transpose(0, 2, 1)
                    .reshape(ng * NQUART * 16, i16c)
                    .reshape(ng * NQUART, 16, i16c)
                    .transpose(1, 0, 2).reshape(16, ng * NQUART * i16c),
                    (8, 1)))
        in_maps.append({
            "x": xf,
            "gidx": idx16,
            "tgt": np.ascontiguousarray(tcore),
        })
    return tq, in_maps, blk, slot


def _run(x, edge_index, trace=False):
    from concourse.bass_utils import run_bass_kernel_spmd

    tq, in_maps, blk, slot = _host_prep(x, edge_index)
    key = ("prog", tq, MM_DT)
    if key not in _CACHE:
        nc_ = _build_program(N_NODES, N_FEAT, NB, tq, GB, NQ, MM_DT)
        nc_.finalize()
        _CACHE[key] = nc_
    nc = _CACHE[key]
    res = run_bass_kernel_spmd(
        nc, in_maps, core_ids=list(range(NCORES)), trace=trace)

    outs = [np.asarray(r["out"]) for r in res.results]
    out_full = np.empty((N_NODES, N_FEAT), np.float32)
    cores = blk // NB
    rows = (blk % NB) * P + slot
    for c in range(NCORES):
        m = cores == c
        out_full[m] = outs[c][rows[m]]
    return out_full, res


def kernel(**inputs):
    out, _ = _run(inputs["x"], inputs["edge_index"], trace=False)
    return out



# revision 12
# speedup vs baseline: 2.1423x; 2.1423x over previous
"""GNN message-passing (std aggregator) on 8 TRN2 NeuronCores.

Math per target node: count, S1 = sum x[src], S2 = sum x[src]^2;
mean = S1/max(count,eps); var = S2/count - mean^2;
std = sqrt(max(var,0)), zeroed where count <= 1.

Strategy: shard TARGET nodes across cores (no collectives). Host packs nodes
into 128-bin blocks balanced by in-degree (serpentine deal), buckets edges by
(block, src-quarter) with uniform tile capacity tq per (block,quarter) so one
NEFF serves all cores. Device per core, per group of GB blocks:
  - 4x dma_gather (one per src quarter of x; int16 idx < 25000) pulls
    x[src] rows into SBUF in quarter-major column order,
  - ACT builds rhs tiles [x | x^2 | 1] (cast to MM dtype),
  - DVE builds 4-wide one-hot tiles (slot-vs-iota is_equal),
  - PE matmul-accumulates [128 bins x 129] = [S1 | S2 | count] in PSUM,
  - small DVE/ACT finishing pass computes std, DMA out per block.
"""

import numpy as np

N_NODES = 100000
N_FEAT = 64
N_EDGES = 1600000
P = 128
NCORES = 8
NB = 98                 # blocks per core
NBLK = NCORES * NB      # 784
GB = 7                  # blocks per group; 98 = 14*7
NQUART = 4
NQ = N_NODES // NQUART  # rows per src quarter (25000 < 32768 for int16 idx)
EPS = 1e-8
MM_DT = "bfloat16"      # "float32" | "bfloat16" for matmul operands

_CACHE = {}


def _build_program(n_nodes, f, nb, tq, gb, nq, mm_dt):
    import concourse.bass as bass
    import concourse.bacc as bacc
    import concourse.mybir as mybir
    import concourse.tile as tile

    F32 = mybir.dt.float32
    I16 = mybir.dt.int16
    MDT = getattr(mybir.dt, mm_dt)
    AO = mybir.AluOpType
    AF = mybir.ActivationFunctionType

    t = NQUART * tq            # tiles (columns) per block
    W = 2 * f + 1              # 129
    C = nb * t                 # total columns per core
    gcols = gb * t             # columns per group
    qcols = gb * tq            # columns per (group, quarter)
    ng = nb // gb
    nidx = qcols * P           # indices per gather
    i16c = nidx // 16          # idx16 cols per gather

    nc = bacc.Bacc(num_swdge_queues=4)
    xd = nc.declare_dram_parameter("x", [n_nodes, f], F32, isOutput=False)
    gidxd = nc.declare_dram_parameter(
        "gidx", [P, ng * NQUART * i16c], I16, isOutput=False)
    tgtd = nc.declare_dram_parameter("tgt", [P, C], F32, isOutput=False)
    outd = nc.declare_dram_parameter("out", [nb * P, f], F32, isOutput=True)

    with tile.TileContext(nc) as tc:
        with (
            tc.tile_pool(name="const", bufs=1) as constp,
            tc.tile_pool(name="io", bufs=2) as iop,
            tc.tile_pool(name="msg", bufs=2) as msgp,
            tc.tile_pool(name="oh", bufs=6) as ohp,
            tc.tile_pool(name="fin", bufs=4) as finp,
            tc.tile_pool(name="ov", bufs=4) as ovp,
            tc.tile_pool(name="ps", bufs=8, space="PSUM") as psump,
        ):
            # iota [128, 128]: value = column index (bf16; 0..127 exact)
            iota1 = constp.tile([P, P], MDT)
            nc.gpsimd.iota(iota1[:], pattern=[[1, P]], base=0,
                           channel_multiplier=0,
                           allow_small_or_imprecise_dtypes=True)

            for g in range(ng):
                idx = iop.tile([P, NQUART * i16c], I16, tag="idx")
                tg = iop.tile([P, gcols], F32, tag="tg")
                nc.sync.dma_start(
                    out=idx[:],
                    in_=gidxd[:, g * NQUART * i16c:(g + 1) * NQUART * i16c])
                nc.sync.dma_start(
                    out=tg[:], in_=tgtd[:, g * gcols:(g + 1) * gcols])

                gbuf = msgp.tile([P, gcols * f], F32, tag="g")
                g3 = gbuf[:].rearrange("p (c e) -> p c e", e=f)
                for qq in range(NQUART):
                    nc.gpsimd.dma_gather(
                        out_ap=g3[:, qq * qcols:(qq + 1) * qcols, :],
                        in_ap=xd[qq * nq:(qq + 1) * nq, :],
                        idxs_ap=idx[:, qq * i16c:(qq + 1) * i16c],
                        num_idxs=nidx,
                        num_idxs_reg=nidx,
                        elem_size=f,
                        single_packet=False,
                        queue_num=qq,
                    )
                sqx = msgp.tile([P, gcols * W], MDT, tag="sqx")
                s3 = sqx[:].rearrange("p (c w) -> p c w", w=W)
                nc.scalar.activation(out=s3[:, :, 0:f], in_=g3[:, :, :],
                                     func=AF.Copy)
                nc.scalar.square(out=s3[:, :, f:2 * f], in_=g3[:, :, :])
                nc.scalar.activation(out=s3[:, :, 2 * f:W], in_=g3[:, :, 0:1],
                                     func=AF.Copy, bias=1.0, scale=0.0)

                pss = [psump.tile([P, W], F32, tag="ps", name=f"ps_{g}_{bl}")
                       for bl in range(gb)]
                for cl in range(gcols):
                    oh = ohp.tile([P, P], MDT)
                    nc.vector.tensor_scalar(
                        out=oh[:], in0=iota1[:],
                        scalar1=tg[:, cl:cl + 1], scalar2=None,
                        op0=AO.is_equal)
                    qq = cl // qcols
                    r = cl % qcols
                    bl = r // tq
                    j = r % tq
                    nc.tensor.matmul(
                        out=pss[bl][:],
                        lhsT=oh[:],
                        rhs=sqx[:, cl * W:(cl + 1) * W],
                        start=(qq == 0 and j == 0),
                        stop=(qq == NQUART - 1 and j == tq - 1),
                    )
                for bl in range(gb):
                    b = g * gb + bl
                    ps = pss[bl]
                    cnt = finp.tile([P, 1], F32, tag="cnt")
                    nc.vector.tensor_scalar(
                        out=cnt[:], in0=ps[:, 2 * f:W],
                        scalar1=float(EPS), scalar2=None, op0=AO.max)
                    rec = finp.tile([P, 1], F32, tag="rec")
                    nc.vector.reciprocal(out=rec[:], in_=cnt[:])
                    mean = finp.tile([P, f], F32, tag="mean")
                    nc.vector.tensor_scalar_mul(
                        out=mean[:], in0=ps[:, 0:f], scalar1=rec[:])
                    ex2 = finp.tile([P, f], F32, tag="ex2")
                    nc.vector.tensor_scalar_mul(
                        out=ex2[:], in0=ps[:, f:2 * f], scalar1=rec[:])
                    var = finp.tile([P, f], F32, tag="var")
                    nc.vector.tensor_tensor(
                        out=var[:], in0=mean[:], in1=mean[:], op=AO.mult)
                    nc.vector.tensor_tensor(
                        out=var[:], in0=ex2[:], in1=var[:], op=AO.subtract)
                    nc.vector.tensor_scalar(
                        out=var[:], in0=var[:], scalar1=0.0, scalar2=None,
                        op0=AO.max)
                    std = ovp.tile([P, f], F32, tag="std")
                    nc.scalar.sqrt(out=std[:], in_=var[:])
                    mask = finp.tile([P, 1], F32, tag="mask")
                    nc.vector.tensor_scalar(
                        out=mask[:], in0=ps[:, 2 * f:W],
                        scalar1=1.5, scalar2=None, op0=AO.is_gt)
                    nc.vector.tensor_scalar_mul(
                        out=std[:], in0=std[:], scalar1=mask[:])
                    nc.sync.dma_start(
                        out=outd[b * P:(b + 1) * P, :], in_=std[:])
    return nc


def _host_prep(x, edge_index):
    src = np.asarray(edge_index[0], dtype=np.int64)
    tgt = np.asarray(edge_index[1], dtype=np.int64)
    n_edges = src.shape[0]
    counts = np.bincount(tgt, minlength=N_NODES)

    # serpentine deal of count-sorted nodes into NBLK blocks of <=128 slots
    order = np.argsort(-counts, kind="stable")
    ranks = np.arange(N_NODES)
    rounds = ranks // NBLK
    pos = ranks % NBLK
    blk_of_rank = np.where(rounds % 2 == 0, pos, NBLK - 1 - pos)
    blk = np.empty(N_NODES, np.int64)
    slot = np.empty(N_NODES, np.int64)
    blk[order] = blk_of_rank
    slot[order] = rounds
    assert slot.max() < P

    eb = blk[tgt]                      # edge -> block
    eq = src // NQ                     # edge -> src quarter
    es = slot[tgt]                     # edge -> slot in block
    seg = eb * NQUART + eq             # edge -> (block, quarter) segment
    segsums = np.bincount(seg, minlength=NBLK * NQUART)
    tq = int(np.ceil(segsums.max() / P))
    cap = tq * P

    order_e = np.argsort(seg, kind="stable")
    segs = seg[order_e]
    starts = np.zeros(NBLK * NQUART, np.int64)
    np.cumsum(segsums[:-1], out=starts[1:])
    within = np.arange(n_edges) - starts[segs]
    flat = segs * cap + within

    gidxq = np.zeros((NBLK, NQUART, cap), np.int16)
    tgtq = np.full((NBLK, NQUART, cap), -1.0, np.float32)
    gidxq.reshape(-1)[flat] = (src[order_e] % NQ).astype(np.int16)
    tgtq.reshape(-1)[flat] = es[order_e].astype(np.float32)

    xf = np.ascontiguousarray(np.asarray(x, dtype=np.float32))
    ng = NB // GB
    i16c = GB * cap // 16

    in_maps = []
    for c in range(NCORES):
        tb = tgtq[c * NB:(c + 1) * NB]          # [NB, 4, cap]
        gi = gidxq[c * NB:(c + 1) * NB]
        # tgt columns: (group, quarter, block, tile) -> [P, C]
        tcore = (tb.reshape(ng, GB, NQUART, cap)
                 .transpose(0, 2, 1, 3)          # [ng, 4, GB, cap]
                 .reshape(ng * NQUART * GB * tq, P).T)
        # idx16: per (group, quarter): stream of GB*cap idxs wrapped %16
        gs = (gi.reshape(ng, GB, NQUART, cap)
              .transpose(0, 2, 1, 3)             # [ng, 4, GB, cap]
              .reshape(ng * NQUART, GB * cap))   # per-gather streams
        idx16 = np.ascontiguousarray(
            np.tile(gs.reshape(ng * NQUART, i16c, 16).transpose(0, 2, 1)
                    .reshape(ng * NQUART * 16, i16c)
                    .reshape(ng * NQUART, 16, i16c)
                    .transpose(1, 0, 2).reshape(16, ng * NQUART * i16c),
                    (8, 1)))
        in_maps.append({
            "x": xf,
            "gidx": idx16,
            "tgt": np.ascontiguousarray(tcore),
        })
    return tq, in_maps, blk, slot


def _run(x, edge_index, trace=False):
    from concourse.bass_utils import run_bass_kernel_spmd

    tq, in_maps, blk, slot = _host_prep(x, edge_index)
    key = ("prog", tq, MM_DT)
    if key not in _CACHE:
        nc_ = _build_program(N_NODES, N_FEAT, NB, tq, GB, NQ, MM_DT)
        nc_.finalize()
        _CACHE[key] = nc_
    nc = _CACHE[key]
    res = run_bass_kernel_spmd(
        nc, in_maps, core_ids=list(range(NCORES)), trace=trace)

    outs = [np.asarray(r["out"]) for r in res.results]
    out_full = np.empty((N_NODES, N_FEAT), np.float32)
    cores = blk // NB
    rows = (blk % NB) * P + slot
    for c in range(NCORES):
        m = cores == c
        out_full[m] = outs[c][rows[m]]
    return out_full, res


def kernel(**inputs):
    out, _ = _run(inputs["x"], inputs["edge_index"], trace=False)
    return out



# revision 16
# speedup vs baseline: 2.9723x; 1.3874x over previous
"""GNN message-passing (std aggregator) on 8 TRN2 NeuronCores.

Math per target node: count, S1 = sum x[src], S2 = sum x[src]^2;
mean = S1/count; var = S2/count - mean^2; std = sqrt(max(var,0)),
zeroed where count <= 1 (host folds the mask into rec = 1/count).

Strategy: shard TARGET nodes across cores (no collectives). Host packs nodes
into 128-bin blocks balanced by in-degree (serpentine deal). Per block, edges
are sorted by src and split into NRUN equal-count runs; run k of all blocks is
gathered from a 32768-row window of x (base_k chosen from the data) so int16
gather indices suffice with ~0 capacity padding (tq = ceil(maxrun/128)).
Device per core, per group of GB blocks:
  - 4x dma_gather (one per run; one SWDGE queue each so the 4 Q7 core-pairs
    generate descriptors in parallel) pulls x[src] rows into SBUF,
  - ACT builds rhs tiles [x | x^2] (bf16),
  - DVE builds one-hot tiles (fp8e4) via per-partition tensor_scalar is_equal,
  - PE matmul-accumulates [128 slots x 128] = [S1 | S2] into one PSUM tile
    per group ([128, GB*128]),
  - batched finishing: mean/ex2 via host-sent rec table, var, sqrt, one DMA
    out per group.
"""

import numpy as np

N_NODES = 100000
N_FEAT = 64
N_EDGES = 1600000
P = 128
NCORES = 8
NB = 98                 # blocks per core
NBLK = NCORES * NB      # 784
GB = 7                  # blocks per group; 98 = 14*7
NRUN = 4                # equal-count src-sorted runs per block
WIN = 32768             # gather window rows (int16 idx space)
MM_DT = "bfloat16"      # rhs dtype for matmul
OH_DT = "float8e4"      # one-hot dtype (0/1 exact)

_CACHE = {}


def _build_program(n_nodes, f, nb, tq, gb, bases, mm_dt, oh_dt):
    import concourse.bass as bass
    import concourse.bacc as bacc
    import concourse.mybir as mybir
    import concourse.tile as tile

    F32 = mybir.dt.float32
    I16 = mybir.dt.int16
    MDT = getattr(mybir.dt, mm_dt)
    ODT = getattr(mybir.dt, oh_dt)
    AO = mybir.AluOpType
    AF = mybir.ActivationFunctionType

    t = NRUN * tq              # tiles (columns) per block
    W = 2 * f                  # 128
    C = nb * t                 # total columns per core
    gcols = gb * t             # columns per group
    rcols = gb * tq            # columns per (group, run)
    ng = nb // gb
    nidx = rcols * P           # indices per gather
    i16c = nidx // 16          # idx16 cols per gather

    nc = bacc.Bacc(num_swdge_queues=4)
    xd = nc.declare_dram_parameter("x", [n_nodes, f], F32, isOutput=False)
    gidxd = nc.declare_dram_parameter(
        "gidx", [P, ng * NRUN * i16c], I16, isOutput=False)
    tgtd = nc.declare_dram_parameter("tgt", [P, C], F32, isOutput=False)
    recd = nc.declare_dram_parameter("rec", [P, nb], F32, isOutput=False)
    outd = nc.declare_dram_parameter("out", [nb * P, f], F32, isOutput=True)

    with tile.TileContext(nc) as tc:
        with (
            tc.tile_pool(name="const", bufs=1) as constp,
            tc.tile_pool(name="io", bufs=2) as iop,
            tc.tile_pool(name="msg", bufs=2) as msgp,
            tc.tile_pool(name="oh", bufs=6) as ohp,
            tc.tile_pool(name="fin", bufs=2) as finp,
            tc.tile_pool(name="ov", bufs=2) as ovp,
            tc.tile_pool(name="ps", bufs=8, space="PSUM") as psump,
        ):
            # iota [128, 128]: value = column index (bf16; 0..127 exact)
            iota1 = constp.tile([P, P], MDT)
            nc.gpsimd.iota(iota1[:], pattern=[[1, P]], base=0,
                           channel_multiplier=0,
                           allow_small_or_imprecise_dtypes=True)
            recb = constp.tile([P, nb], F32)
            nc.sync.dma_start(out=recb[:], in_=recd[:, :])

            for g in range(ng):
                idx = iop.tile([P, NRUN * i16c], I16, tag="idx")
                tg = iop.tile([P, gcols], F32, tag="tg")
                nc.sync.dma_start(
                    out=idx[:],
                    in_=gidxd[:, g * NRUN * i16c:(g + 1) * NRUN * i16c])
                nc.sync.dma_start(
                    out=tg[:], in_=tgtd[:, g * gcols:(g + 1) * gcols])

                gbuf = msgp.tile([P, gcols * f], F32, tag="g")
                g3 = gbuf[:].rearrange("p (c e) -> p c e", e=f)
                for k in range(NRUN):
                    nc.gpsimd.dma_gather(
                        out_ap=g3[:, k * rcols:(k + 1) * rcols, :],
                        in_ap=xd[bases[k]:bases[k] + WIN, :],
                        idxs_ap=idx[:, k * i16c:(k + 1) * i16c],
                        num_idxs=nidx,
                        num_idxs_reg=nidx,
                        elem_size=f,
                        single_packet=False,
                        queue_num=k,
                    )
                sqx = msgp.tile([P, gcols * W], MDT, tag="sqx")
                s3 = sqx[:].rearrange("p (c w) -> p c w", w=W)
                nc.scalar.activation(out=s3[:, :, 0:f], in_=g3[:, :, :],
                                     func=AF.Copy)
                nc.scalar.square(out=s3[:, :, f:W], in_=g3[:, :, :])

                pss = [psump.tile([P, W], F32, tag="ps", name=f"ps_{g}_{bl}")
                       for bl in range(gb)]
                for cl in range(gcols):
                    oh = ohp.tile([P, P], ODT)
                    nc.vector.tensor_scalar(
                        out=oh[:], in0=iota1[:],
                        scalar1=tg[:, cl:cl + 1], scalar2=None,
                        op0=AO.is_equal)
                    k = cl // rcols
                    r = cl % rcols
                    bl = r // tq
                    j = r % tq
                    nc.tensor.matmul(
                        out=pss[bl][:],
                        lhsT=oh[:],
                        rhs=sqx[:, cl * W:(cl + 1) * W],
                        start=(k == 0 and j == 0),
                        stop=(k == NRUN - 1 and j == tq - 1),
                    )
                # finishing for the whole group: [S1|S2] -> std
                # psum -> sbuf copy fused with the 1/count multiply
                me = finp.tile([P, gb * W], F32, tag="me")
                for bl in range(gb):
                    nc.vector.tensor_scalar_mul(
                        out=me[:, bl * W:(bl + 1) * W], in0=pss[bl][:],
                        scalar1=recb[:, g * gb + bl:g * gb + bl + 1])
                m3 = me[:].rearrange("p (c w) -> p c w", w=W)
                sqm = finp.tile([P, gb * f], F32, tag="sqm")
                q3 = sqm[:].rearrange("p (c e) -> p c e", e=f)
                nc.scalar.square(out=q3[:, :, :], in_=m3[:, :, 0:f])
                va = finp.tile([P, gb * f], F32, tag="va")
                v3 = va[:].rearrange("p (c e) -> p c e", e=f)
                nc.vector.tensor_tensor(
                    out=v3[:, :, :], in0=m3[:, :, f:W], in1=q3[:, :, :],
                    op=AO.subtract)
                nc.vector.tensor_scalar(
                    out=va[:], in0=va[:], scalar1=0.0, scalar2=None,
                    op0=AO.max)
                so = ovp.tile([P, gb * f], F32, tag="so")
                nc.scalar.sqrt(out=so[:], in_=va[:])
                nc.sync.dma_start(
                    out=outd[g * gb * P:(g + 1) * gb * P, :]
                        .rearrange("(c p) e -> p c e", p=P),
                    in_=so[:].rearrange("p (c e) -> p c e", e=f))
    return nc


def _host_prep(x, edge_index):
    src = np.asarray(edge_index[0], dtype=np.int64)
    tgt = np.asarray(edge_index[1], dtype=np.int64)
    n_edges = src.shape[0]
    counts = np.bincount(tgt, minlength=N_NODES)

    # serpentine deal of count-sorted nodes into NBLK blocks of <=128 slots
    order = np.argsort(-counts, kind="stable")
    ranks = np.arange(N_NODES)
    rounds = ranks // NBLK
    pos = ranks % NBLK
    blk_of_rank = np.where(rounds % 2 == 0, pos, NBLK - 1 - pos)
    blk = np.empty(N_NODES, np.int64)
    slot = np.empty(N_NODES, np.int64)
    blk[order] = blk_of_rank
    slot[order] = rounds
    assert slot.max() < P

    eb = blk[tgt]                      # edge -> block
    es = slot[tgt]                     # edge -> slot in block
    bc = np.bincount(eb, minlength=NBLK)
    starts_b = np.zeros(NBLK, np.int64)
    np.cumsum(bc[:-1], out=starts_b[1:])

    # per block: sort edges by src, split into NRUN equal-count runs
    order_e = np.lexsort((src, eb))
    sb = src[order_e]
    r = np.arange(n_edges) - starts_b[eb[order_e]]   # pos within block
    nb_of_e = bc[eb[order_e]]                        # block size per edge
    k = (r * NRUN) // nb_of_e                        # run of edge
    w = r - (k * nb_of_e + NRUN - 1) // NRUN         # pos within run
    run_sizes = np.bincount(eb[order_e] * NRUN + k, minlength=NBLK * NRUN)
    tq = int(np.ceil(run_sizes.max() / P))
    cap = tq * P
    assert (w >= 0).all() and (w < cap).all()

    # static gather-window bases per run
    bases = []
    for kk in range(NRUN):
        m = k == kk
        lo = int(sb[m].min())
        hi = int(sb[m].max())
        base = min(lo, N_NODES - WIN)
        assert base >= 0 and hi - base < WIN, (kk, lo, hi, base)
        bases.append(base)
    bases = tuple(bases)

    flat = (eb[order_e] * NRUN + k) * cap + w
    gidxq = np.zeros((NBLK, NRUN, cap), np.int16)
    tgtq = np.full((NBLK, NRUN, cap), -1.0, np.float32)
    gidxq.reshape(-1)[flat] = (sb - np.asarray(bases)[k]).astype(np.int16)
    tgtq.reshape(-1)[flat] = es[order_e].astype(np.float32)

    # rec table: [slot, block] -> 1/count (0 where count<=1 or empty slot)
    rec = np.where(counts >= 2, 1.0 / np.maximum(counts, 1), 0.0)
    recq = np.zeros((NBLK, P), np.float32)
    recq[blk, slot] = rec

    xf = np.ascontiguousarray(np.asarray(x, dtype=np.float32))
    ng = NB // GB
    i16c = GB * cap // 16

    in_maps = []
    for c in range(NCORES):
        tb = tgtq[c * NB:(c + 1) * NB]          # [NB, NRUN, cap]
        gi = gidxq[c * NB:(c + 1) * NB]
        # tgt columns: (group, run, block, tile) -> [P, C]
        tcore = (tb.reshape(ng, GB, NRUN, cap)
                 .transpose(0, 2, 1, 3)          # [ng, NRUN, GB, cap]
                 .reshape(ng * NRUN * GB * tq, P).T)
        # idx16: per (group, run): stream of GB*cap idxs wrapped %16
        gs = (gi.reshape(ng, GB, NRUN, cap)
              .transpose(0, 2, 1, 3)             # [ng, NRUN, GB, cap]
              .reshape(ng * NRUN, GB * cap))     # per-gather streams
        idx16 = np.ascontiguousarray(
            np.tile(gs.reshape(ng * NRUN, i16c, 16).transpose(0, 2, 1)
                    .reshape(ng * NRUN * 16, i16c)
                    .reshape(ng * NRUN, 16, i16c)
                    .transpose(1, 0, 2).reshape(16, ng * NRUN * i16c),
                    (8, 1)))
        in_maps.append({
            "x": xf,
            "gidx": idx16,
            "tgt": np.ascontiguousarray(tcore),
            "rec": np.ascontiguousarray(recq[c * NB:(c + 1) * NB].T),
        })
    return tq, bases, in_maps, blk, slot


def _run(x, edge_index, trace=False):
    from concourse.bass_utils import run_bass_kernel_spmd

    tq, bases, in_maps, blk, slot = _host_prep(x, edge_index)
    key = ("prog", tq, bases, MM_DT, OH_DT)
    if key not in _CACHE:
        nc_ = _build_program(N_NODES, N_FEAT, NB, tq, GB, bases, MM_DT, OH_DT)
        nc_.finalize()
        _CACHE[key] = nc_
    nc = _CACHE[key]
    res = run_bass_kernel_spmd(
        nc, in_maps, core_ids=list(range(NCORES)), trace=trace)

    outs = [np.asarray(r["out"]) for r in res.results]
    out_full = np.empty((N_NODES, N_FEAT), np.float32)
    cores = blk // NB
    rows = (blk % NB) * P + slot
    for c in range(NCORES):
        m = cores == c
        out_full[m] = outs[c][rows[m]]
    return out_full, res


def kernel(**inputs):
    out, _ = _run(inputs["x"], inputs["edge_index"], trace=False)
    return out


# revision 17
# speedup vs baseline: 4.2418x; 1.4271x over previous
"""GNN message-passing (std aggregator) on 8 TRN2 NeuronCores.

Math per target node: count, S1 = sum x[src], S2 = sum x[src]^2;
mean = S1/count; var = S2/count - mean^2; std = sqrt(max(var,0)),
zeroed where count <= 1 (host folds the mask into rec = 1/count).

Strategy: shard TARGET nodes across cores (no collectives). Host packs nodes
into 128-bin blocks balanced by in-degree (serpentine deal). Per block, edges
are sorted by src and split into NRUN equal-count runs; run k of all blocks is
gathered from a 32768-row window (base_k from the data) so int16 gather
indices suffice with ~0 capacity padding. Host also ships (a) an interleaved
bf16 xx = [x | x^2] table so one dma_gather descriptor (256B) fetches a
ready-made rhs row, and (b) the per-edge one-hot routing tiles pre-encoded in
fp8e4 so no engine has to build them. Device per core, per group of GB blocks:
  - 4x dma_gather (one per run; one SWDGE queue each so the 4 Q7 core-pairs
    generate descriptors in parallel) pulls xx[src] rows into SBUF as the
    matmul rhs,
  - a sequential DMA loads the fp8 one-hot tiles,
  - PE matmul-accumulates [128 slots x 128] = [S1 | S2] in per-block PSUM
    banks (zero-region = 2KB bank, so one open group per bank),
  - finishing: per-block PSUM->SBUF copy fused with the 1/count multiply,
    then batched var/sqrt and one DMA out per group.
"""

import numpy as np

N_NODES = 100000
N_FEAT = 64
N_EDGES = 1600000
P = 128
NCORES = 8
NB = 98                 # blocks per core
NBLK = NCORES * NB      # 784
GB = 7                  # blocks per group; 98 = 14*7
NRUN = 4                # equal-count src-sorted runs per block
WIN = 32768             # gather window rows (int16 idx space)
MM_DT = "bfloat16"      # rhs dtype for matmul
OH_DT = "float8e4"      # one-hot dtype (0/1 exact)
FP8_ONE = 0x38          # float8_e4m3 bit pattern of 1.0

_CACHE = {}


def _build_program(n_nodes, f, nb, tq, gb, bases, mm_dt, oh_dt):
    import concourse.bacc as bacc
    import concourse.mybir as mybir
    import concourse.tile as tile

    F32 = mybir.dt.float32
    I16 = mybir.dt.int16
    MDT = getattr(mybir.dt, mm_dt)
    ODT = getattr(mybir.dt, oh_dt)
    AO = mybir.AluOpType

    t = NRUN * tq              # tiles (columns) per block
    W = 2 * f                  # 128
    C = nb * t                 # total columns per core
    gcols = gb * t             # columns per group
    rcols = gb * tq            # columns per (group, run)
    ng = nb // gb
    nidx = rcols * P           # indices per gather
    i16c = nidx // 16          # idx16 cols per gather

    nc = bacc.Bacc(num_swdge_queues=4)
    xxd = nc.declare_dram_parameter("xx", [n_nodes, W], MDT, isOutput=False)
    gidxd = nc.declare_dram_parameter(
        "gidx", [P, ng * NRUN * i16c], I16, isOutput=False)
    ohd = nc.declare_dram_parameter("oh", [P, C * P], ODT, isOutput=False)
    recd = nc.declare_dram_parameter("rec", [P, nb], F32, isOutput=False)
    outd = nc.declare_dram_parameter("out", [nb * P, f], F32, isOutput=True)

    with tile.TileContext(nc) as tc:
        with (
            tc.tile_pool(name="const", bufs=1) as constp,
            tc.tile_pool(name="io", bufs=2) as iop,
            tc.tile_pool(name="msg", bufs=2) as msgp,
            tc.tile_pool(name="fin", bufs=2) as finp,
            tc.tile_pool(name="ov", bufs=2) as ovp,
            tc.tile_pool(name="ps", bufs=8, space="PSUM") as psump,
        ):
            recb = constp.tile([P, nb], F32)
            nc.sync.dma_start(out=recb[:], in_=recd[:, :])

            for g in range(ng):
                idx = iop.tile([P, NRUN * i16c], I16, tag="idx")
                nc.sync.dma_start(
                    out=idx[:],
                    in_=gidxd[:, g * NRUN * i16c:(g + 1) * NRUN * i16c])
                ohg = msgp.tile([P, gcols * P], ODT, tag="ohg")
                nc.sync.dma_start(
                    out=ohg[:], in_=ohd[:, g * gcols * P:(g + 1) * gcols * P])

                sqx = msgp.tile([P, gcols * W], MDT, tag="sqx")
                s3 = sqx[:].rearrange("p (c w) -> p c w", w=W)
                for k in range(NRUN):
                    nc.gpsimd.dma_gather(
                        out_ap=s3[:, k * rcols:(k + 1) * rcols, :],
                        in_ap=xxd[bases[k]:bases[k] + WIN, :],
                        idxs_ap=idx[:, k * i16c:(k + 1) * i16c],
                        num_idxs=nidx,
                        num_idxs_reg=nidx,
                        elem_size=W,
                        single_packet=False,
                        queue_num=k,
                    )
                pss = [psump.tile([P, W], F32, tag="ps", name=f"ps_{g}_{bl}")
                       for bl in range(gb)]
                for cl in range(gcols):
                    k = cl // rcols
                    r = cl % rcols
                    bl = r // tq
                    j = r % tq
                    nc.tensor.matmul(
                        out=pss[bl][:],
                        lhsT=ohg[:, cl * P:(cl + 1) * P],
                        rhs=sqx[:, cl * W:(cl + 1) * W],
                        start=(k == 0 and j == 0),
                        stop=(k == NRUN - 1 and j == tq - 1),
                    )
                # finishing for the whole group: [S1|S2] -> std
                # psum -> sbuf copy fused with the 1/count multiply
                me = finp.tile([P, gb * W], F32, tag="me")
                for bl in range(gb):
                    nc.vector.tensor_scalar_mul(
                        out=me[:, bl * W:(bl + 1) * W], in0=pss[bl][:],
                        scalar1=recb[:, g * gb + bl:g * gb + bl + 1])
                m3 = me[:].rearrange("p (c w) -> p c w", w=W)
                sqm = finp.tile([P, gb * f], F32, tag="sqm")
                q3 = sqm[:].rearrange("p (c e) -> p c e", e=f)
                nc.scalar.square(out=q3[:, :, :], in_=m3[:, :, 0:f])
                va = finp.tile([P, gb * f], F32, tag="va")
                v3 = va[:].rearrange("p (c e) -> p c e", e=f)
                nc.vector.tensor_tensor(
                    out=v3[:, :, :], in0=m3[:, :, f:W], in1=q3[:, :, :],
                    op=AO.subtract)
                nc.vector.tensor_scalar(
                    out=va[:], in0=va[:], scalar1=0.0, scalar2=None,
                    op0=AO.max)
                so = ovp.tile([P, gb * f], F32, tag="so")
                nc.scalar.sqrt(out=so[:], in_=va[:])
                nc.sync.dma_start(
                    out=outd[g * gb * P:(g + 1) * gb * P, :]
                        .rearrange("(c p) e -> p c e", p=P),
                    in_=so[:].rearrange("p (c e) -> p c e", e=f))
    return nc


def _host_prep(x, edge_index):
    import ml_dtypes

    src = np.asarray(edge_index[0], dtype=np.int64)
    tgt = np.asarray(edge_index[1], dtype=np.int64)
    n_edges = src.shape[0]
    counts = np.bincount(tgt, minlength=N_NODES)

    # serpentine deal of count-sorted nodes into NBLK blocks of <=128 slots
    order = np.argsort(-counts, kind="stable")
    ranks = np.arange(N_NODES)
    rounds = ranks // NBLK
    pos = ranks % NBLK
    blk_of_rank = np.where(rounds % 2 == 0, pos, NBLK - 1 - pos)
    blk = np.empty(N_NODES, np.int64)
    slot = np.empty(N_NODES, np.int64)
    blk[order] = blk_of_rank
    slot[order] = rounds
    assert slot.max() < P

    eb = blk[tgt]                      # edge -> block
    es = slot[tgt]                     # edge -> slot in block
    bc = np.bincount(eb, minlength=NBLK)
    starts_b = np.zeros(NBLK, np.int64)
    np.cumsum(bc[:-1], out=starts_b[1:])

    # per block: sort edges by src, split into NRUN equal-count runs
    order_e = np.lexsort((src, eb))
    sb = src[order_e]
    ebo = eb[order_e]
    eso = es[order_e]
    r = np.arange(n_edges) - starts_b[ebo]           # pos within block
    nb_of_e = bc[ebo]                                # block size per edge
    k = (r * NRUN) // nb_of_e                        # run of edge
    w = r - (k * nb_of_e + NRUN - 1) // NRUN         # pos within run
    run_sizes = np.bincount(ebo * NRUN + k, minlength=NBLK * NRUN)
    tq = int(np.ceil(run_sizes.max() / P))
    cap = tq * P
    assert (w >= 0).all() and (w < cap).all()

    # static gather-window bases per run
    bases = []
    for kk in range(NRUN):
        m = k == kk
        lo = int(sb[m].min())
        hi = int(sb[m].max())
        base = min(lo, N_NODES - WIN)
        assert base >= 0 and hi - base < WIN, (kk, lo, hi, base)
        bases.append(base)
    bases = tuple(bases)

    flat = (ebo * NRUN + k) * cap + w
    gidxq = np.zeros((NBLK, NRUN, cap), np.int16)
    gidxq.reshape(-1)[flat] = (sb - np.asarray(bases)[k]).astype(np.int16)

    # rec table: [slot, block] -> 1/count (0 where count<=1 or empty slot)
    rec = np.where(counts >= 2, 1.0 / np.maximum(counts, 1), 0.0)
    recq = np.zeros((NBLK, P), np.float32)
    recq[blk, slot] = rec.astype(np.float32)

    # interleaved rhs table [x | x^2] in bf16
    xf = np.asarray(x, dtype=np.float32)
    xx = np.empty((N_NODES, 2 * N_FEAT), ml_dtypes.bfloat16)
    xx[:, :N_FEAT] = xf
    xx[:, N_FEAT:] = xf * xf
    xx = np.ascontiguousarray(xx)

    ng = NB // GB
    i16c = GB * cap // 16
    C = NB * NRUN * tq

    # per-edge one-hot routing tiles, fp8e4: [P, C, P]; 1 at
    # (partition = w%128, col = (g,k,bl,j), slot)
    core_e = ebo // NB
    bloc = ebo % NB
    g_e = bloc // GB
    bl_e = bloc % GB
    j_e = w // P
    p_e = w % P
    cl_e = g_e * (NRUN * GB * tq) + k * (GB * tq) + bl_e * tq + j_e
    flat_oh = p_e * (C * P) + cl_e * P + eso

    in_maps = []
    for c in range(NCORES):
        gi = gidxq[c * NB:(c + 1) * NB]
        # idx16: per (group, run): stream of GB*cap idxs wrapped %16
        gs = (gi.reshape(ng, GB, NRUN, cap)
              .transpose(0, 2, 1, 3)             # [ng, NRUN, GB, cap]
              .reshape(ng * NRUN, GB * cap))     # per-gather streams
        idx16 = np.ascontiguousarray(
            np.tile(gs.reshape(ng * NRUN, i16c, 16).transpose(0, 2, 1)
                    .reshape(ng * NRUN * 16, i16c)
                    .reshape(ng * NRUN, 16, i16c)
                    .transpose(1, 0, 2).reshape(16, ng * NRUN * i16c),
                    (8, 1)))
        oh_u8 = np.zeros(P * C * P, np.uint8)
        oh_u8[flat_oh[core_e == c]] = FP8_ONE
        oh = oh_u8.view(ml_dtypes.float8_e4m3).reshape(P, C * P)
        in_maps.append({
            "xx": xx,
            "gidx": idx16,
            "oh": oh,
            "rec": np.ascontiguousarray(recq[c * NB:(c + 1) * NB].T),
        })
    return tq, bases, in_maps, blk, slot


def _run(x, edge_index, trace=False):
    from concourse.bass_utils import run_bass_kernel_spmd

    tq, bases, in_maps, blk, slot = _host_prep(x, edge_index)
    key = ("prog", tq, bases, MM_DT, OH_DT)
    if key not in _CACHE:
        nc_ = _build_program(N_NODES, N_FEAT, NB, tq, GB, bases, MM_DT, OH_DT)
        nc_.finalize()
        _CACHE[key] = nc_
    nc = _CACHE[key]
    res = run_bass_kernel_spmd(
        nc, in_maps, core_ids=list(range(NCORES)), trace=trace)

    outs = [np.asarray(r["out"]) for r in res.results]
    out_full = np.empty((N_NODES, N_FEAT), np.float32)
    cores = blk // NB
    rows = (blk % NB) * P + slot
    for c in range(NCORES):
        m = cores == c
        out_full[m] = outs[c][rows[m]]
    return out_full, res


def kernel(**inputs):
    out, _ = _run(inputs["x"], inputs["edge_index"], trace=False)
    return out


# revision 18
# speedup vs baseline: 4.9115x; 1.1579x over previous
"""GNN message-passing (std aggregator) on 8 TRN2 NeuronCores.

Math per target node: count, S1 = sum x[src], S2 = sum x[src]^2;
mean = S1/count; var = S2/count - mean^2; std = sqrt(max(var,0)),
zeroed where count <= 1 (host folds the mask into rec = 1/count).

Strategy: shard TARGET nodes across cores (no collectives). Host packs nodes
into 128-bin blocks balanced by in-degree (serpentine deal). Per block, edges
are sorted by src and split into NRUN equal-count runs; run k of all blocks is
gathered from a 32768-row window (base_k from the data) so int16 gather
indices suffice with ~0 capacity padding. Host also ships (a) an interleaved
bf16 xx = [x | x^2] table so one dma_gather descriptor (256B) fetches a
ready-made rhs row, and (b) the per-edge one-hot routing tiles pre-encoded in
fp8e4 so no engine has to build them. Device per core, per group of GB blocks:
  - 4x dma_gather (one per run; one SWDGE queue each so the 4 Q7 core-pairs
    generate descriptors in parallel) pulls xx[src] rows into SBUF as the
    matmul rhs,
  - a sequential DMA loads the fp8 one-hot tiles,
  - PE matmul-accumulates [128 slots x 128] = [S1 | S2] in per-block PSUM
    banks (zero-region = 2KB bank, so one open group per bank),
  - finishing: per-block PSUM->SBUF copy fused with the 1/count multiply,
    then batched var/sqrt and one DMA out per group.
"""

import numpy as np

N_NODES = 100000
N_FEAT = 64
N_EDGES = 1600000
P = 128
NCORES = 8
NB = 98                 # blocks per core
NBLK = NCORES * NB      # 784
GB = 7                  # blocks per group; 98 = 14*7
NRUN = 4                # equal-count src-sorted runs per block
WIN = 32768             # gather window rows (int16 idx space)
MM_DT = "bfloat16"      # rhs dtype for matmul
OH_DT = "float8e4"      # one-hot dtype (0/1 exact)
FP8_ONE = 0x38          # float8_e4m3 bit pattern of 1.0

_CACHE = {}


def _build_program(n_nodes, f, nb, tq, gb, bases, mm_dt, oh_dt):
    import concourse.bacc as bacc
    import concourse.mybir as mybir
    import concourse.tile as tile

    F32 = mybir.dt.float32
    I16 = mybir.dt.int16
    MDT = getattr(mybir.dt, mm_dt)
    ODT = getattr(mybir.dt, oh_dt)
    AO = mybir.AluOpType

    t = NRUN * tq              # tiles (columns) per block
    W = 2 * f                  # 128
    C = nb * t                 # total columns per core
    gcols = gb * t             # columns per group
    rcols = gb * tq            # columns per (group, run)
    ng = nb // gb
    nidx = rcols * P           # indices per gather
    i16c = nidx // 16          # idx16 cols per gather

    nc = bacc.Bacc(num_swdge_queues=4)
    xxd = nc.declare_dram_parameter("xx", [n_nodes, W], MDT, isOutput=False)
    gidxd = nc.declare_dram_parameter(
        "gidx", [P, ng * NRUN * i16c], I16, isOutput=False)
    ohd = nc.declare_dram_parameter("oh", [P, C * P], ODT, isOutput=False)
    recd = nc.declare_dram_parameter("rec", [P, nb], F32, isOutput=False)
    outd = nc.declare_dram_parameter("out", [nb * P, f], F32, isOutput=True)

    with tile.TileContext(nc) as tc:
        with (
            tc.tile_pool(name="const", bufs=1) as constp,
            tc.tile_pool(name="io", bufs=2) as iop,
            tc.tile_pool(name="msg", bufs=2) as msgp,
            tc.tile_pool(name="fin", bufs=2) as finp,
            tc.tile_pool(name="ov", bufs=2) as ovp,
            tc.tile_pool(name="ps", bufs=8, space="PSUM") as psump,
        ):
            # prefetch the whole int16 index table before anything else so
            # the first gathers start as early as possible
            idxall = constp.tile([P, ng * NRUN * i16c], I16)
            nc.sync.dma_start(out=idxall[:], in_=gidxd[:, :])
            recb = constp.tile([P, nb], F32)
            nc.sync.dma_start(out=recb[:], in_=recd[:, :])

            for g in range(ng):
                sqx = msgp.tile([P, gcols * W], MDT, tag="sqx")
                s3 = sqx[:].rearrange("p (c w) -> p c w", w=W)
                for k in range(NRUN):
                    nc.gpsimd.dma_gather(
                        out_ap=s3[:, k * rcols:(k + 1) * rcols, :],
                        in_ap=xxd[bases[k]:bases[k] + WIN, :],
                        idxs_ap=idxall[:, (g * NRUN + k) * i16c:
                                       (g * NRUN + k + 1) * i16c],
                        num_idxs=nidx,
                        num_idxs_reg=nidx,
                        elem_size=W,
                        single_packet=False,
                        queue_num=k,
                    )
                ohg = msgp.tile([P, gcols * P], ODT, tag="ohg")
                nc.sync.dma_start(
                    out=ohg[:], in_=ohd[:, g * gcols * P:(g + 1) * gcols * P])
                pss = [psump.tile([P, W], F32, tag="ps", name=f"ps_{g}_{bl}")
                       for bl in range(gb)]
                for cl in range(gcols):
                    k = cl // rcols
                    r = cl % rcols
                    bl = r // tq
                    j = r % tq
                    nc.tensor.matmul(
                        out=pss[bl][:],
                        lhsT=ohg[:, cl * P:(cl + 1) * P],
                        rhs=sqx[:, cl * W:(cl + 1) * W],
                        start=(k == 0 and j == 0),
                        stop=(k == NRUN - 1 and j == tq - 1),
                    )
                # finishing for the whole group: [S1|S2] -> std
                # psum -> sbuf copy fused with the 1/count multiply
                me = finp.tile([P, gb * W], F32, tag="me")
                for bl in range(gb):
                    nc.vector.tensor_scalar_mul(
                        out=me[:, bl * W:(bl + 1) * W], in0=pss[bl][:],
                        scalar1=recb[:, g * gb + bl:g * gb + bl + 1])
                m3 = me[:].rearrange("p (c w) -> p c w", w=W)
                sqm = finp.tile([P, gb * f], F32, tag="sqm")
                q3 = sqm[:].rearrange("p (c e) -> p c e", e=f)
                nc.scalar.square(out=q3[:, :, :], in_=m3[:, :, 0:f])
                va = finp.tile([P, gb * f], F32, tag="va")
                v3 = va[:].rearrange("p (c e) -> p c e", e=f)
                nc.vector.tensor_tensor(
                    out=v3[:, :, :], in0=m3[:, :, f:W], in1=q3[:, :, :],
                    op=AO.subtract)
                nc.vector.tensor_scalar(
                    out=va[:], in0=va[:], scalar1=0.0, scalar2=None,
                    op0=AO.max)
                so = ovp.tile([P, gb * f], F32, tag="so")
                nc.scalar.sqrt(out=so[:], in_=va[:])
                nc.sync.dma_start(
                    out=outd[g * gb * P:(g + 1) * gb * P, :]
                        .rearrange("(c p) e -> p c e", p=P),
                    in_=so[:].rearrange("p (c e) -> p c e", e=f))
    return nc


def _host_prep(x, edge_index):
    import ml_dtypes

    src = np.asarray(edge_index[0], dtype=np.int64)
    tgt = np.asarray(edge_index[1], dtype=np.int64)
    n_edges = src.shape[0]
    counts = np.bincount(tgt, minlength=N_NODES)

    # serpentine deal of count-sorted nodes into NBLK blocks of <=128 slots
    order = np.argsort(-counts, kind="stable")
    ranks = np.arange(N_NODES)
    rounds = ranks // NBLK
    pos = ranks % NBLK
    blk_of_rank = np.where(rounds % 2 == 0, pos, NBLK - 1 - pos)
    blk = np.empty(N_NODES, np.int64)
    slot = np.empty(N_NODES, np.int64)
    blk[order] = blk_of_rank
    slot[order] = rounds
    assert slot.max() < P

    eb = blk[tgt]                      # edge -> block
    es = slot[tgt]                     # edge -> slot in block
    bc = np.bincount(eb, minlength=NBLK)
    starts_b = np.zeros(NBLK, np.int64)
    np.cumsum(bc[:-1], out=starts_b[1:])

    # per block: sort edges by src, split into NRUN equal-count runs
    order_e = np.lexsort((src, eb))
    sb = src[order_e]
    ebo = eb[order_e]
    eso = es[order_e]
    r = np.arange(n_edges) - starts_b[ebo]           # pos within block
    nb_of_e = bc[ebo]                                # block size per edge
    k = (r * NRUN) // nb_of_e                        # run of edge
    w = r - (k * nb_of_e + NRUN - 1) // NRUN         # pos within run
    run_sizes = np.bincount(ebo * NRUN + k, minlength=NBLK * NRUN)
    tq = int(np.ceil(run_sizes.max() / P))
    cap = tq * P
    assert (w >= 0).all() and (w < cap).all()

    # static gather-window bases per run
    bases = []
    for kk in range(NRUN):
        m = k == kk
        lo = int(sb[m].min())
        hi = int(sb[m].max())
        base = min(lo, N_NODES - WIN)
        assert base >= 0 and hi - base < WIN, (kk, lo, hi, base)
        bases.append(base)
    bases = tuple(bases)

    flat = (ebo * NRUN + k) * cap + w
    gidxq = np.zeros((NBLK, NRUN, cap), np.int16)
    gidxq.reshape(-1)[flat] = (sb - np.asarray(bases)[k]).astype(np.int16)

    # rec table: [slot, block] -> 1/count (0 where count<=1 or empty slot)
    rec = np.where(counts >= 2, 1.0 / np.maximum(counts, 1), 0.0)
    recq = np.zeros((NBLK, P), np.float32)
    recq[blk, slot] = rec.astype(np.float32)

    # interleaved rhs table [x | x^2] in bf16
    xf = np.asarray(x, dtype=np.float32)
    xx = np.empty((N_NODES, 2 * N_FEAT), ml_dtypes.bfloat16)
    xx[:, :N_FEAT] = xf
    xx[:, N_FEAT:] = xf * xf
    xx = np.ascontiguousarray(xx)

    ng = NB // GB
    i16c = GB * cap // 16
    C = NB * NRUN * tq

    # per-edge one-hot routing tiles, fp8e4: [P, C, P]; 1 at
    # (partition = w%128, col = (g,k,bl,j), slot)
    core_e = ebo // NB
    bloc = ebo % NB
    g_e = bloc // GB
    bl_e = bloc % GB
    j_e = w // P
    p_e = w % P
    cl_e = g_e * (NRUN * GB * tq) + k * (GB * tq) + bl_e * tq + j_e
    flat_oh = p_e * (C * P) + cl_e * P + eso

    in_maps = []
    for c in range(NCORES):
        gi = gidxq[c * NB:(c + 1) * NB]
        # idx16: per (group, run): stream of GB*cap idxs wrapped %16
        gs = (gi.reshape(ng, GB, NRUN, cap)
              .transpose(0, 2, 1, 3)             # [ng, NRUN, GB, cap]
              .reshape(ng * NRUN, GB * cap))     # per-gather streams
        idx16 = np.ascontiguousarray(
            np.tile(gs.reshape(ng * NRUN, i16c, 16).transpose(0, 2, 1)
                    .reshape(ng * NRUN * 16, i16c)
                    .reshape(ng * NRUN, 16, i16c)
                    .transpose(1, 0, 2).reshape(16, ng * NRUN * i16c),
                    (8, 1)))
        oh_u8 = np.zeros(P * C * P, np.uint8)
        oh_u8[flat_oh[core_e == c]] = FP8_ONE
        oh = oh_u8.view(ml_dtypes.float8_e4m3).reshape(P, C * P)
        in_maps.append({
            "xx": xx,
            "gidx": idx16,
            "oh": oh,
            "rec": np.ascontiguousarray(recq[c * NB:(c + 1) * NB].T),
        })
    return tq, bases, in_maps, blk, slot


def _run(x, edge_index, trace=False):
    from concourse.bass_utils import run_bass_kernel_spmd

    tq, bases, in_maps, blk, slot = _host_prep(x, edge_index)
    key = ("prog", tq, bases, MM_DT, OH_DT)
    if key not in _CACHE:
        nc_ = _build_program(N_NODES, N_FEAT, NB, tq, GB, bases, MM_DT, OH_DT)
        nc_.finalize()
        _CACHE[key] = nc_
    nc = _CACHE[key]
    res = run_bass_kernel_spmd(
        nc, in_maps, core_ids=list(range(NCORES)), trace=trace)

    outs = [np.asarray(r["out"]) for r in res.results]
    out_full = np.empty((N_NODES, N_FEAT), np.float32)
    cores = blk // NB
    rows = (blk % NB) * P + slot
    for c in range(NCORES):
        m = cores == c
        out_full[m] = outs[c][rows[m]]
    return out_full, res


def kernel(**inputs):
    out, _ = _run(inputs["x"], inputs["edge_index"], trace=False)
    return out
